# revision 5
# baseline (speedup 1.0000x reference)
"""Trainium2 Bass kernel for nn_AttractorState (decay-weighted outer-product state).

Reference computation (per batch b):
    C[b] = sum_t alpha^(S-1-t) * (W @ h_t + bias) outer e_t        (S = 8192)

Refactored to avoid materializing the projection and to keep the big
contraction over t in natural [t, d] layout:
    G[b]  = (w . H[b])^T @ PE[b]          # [d_model=512, d_model=512], w_t = alpha^(S-1-t)
    r[b]  = w^T @ PE[b]                   # [512]
    C[b]  = W @ G[b] + bias outer r[b]    # [d_state=512, d_model=512]

Sharding over 8 NeuronCores: (batch=4) x (seq-half=2).  Each core accumulates
its shard's partial [G; r] with the *global* decay weights baked in (so the
partials just add), then a 2-core ReduceScatter per batch-pair splits the
reduced [G; r] along d: the even core gets d-columns 0:256, the odd core
256:512.  Each core finishes C_half = W @ G_half + b outer r_half on device
and writes a (512, 256) slab; the host reassembles (4, 512, 512).

Matmul operands are cast to bf16 on-chip (DVE/ACT, overlapped with DMA);
accumulation is fp32 in PSUM.
"""

import math
import sys

import numpy as np

for _p in ("/opt/trn_rl_repo", "/opt/trn_rl_repo/concourse"):
    if _p not in sys.path:
        sys.path.append(_p)

# Problem constants (hardcoded per harness contract).
B = 4
S = 8192
D = 512          # d_model
E = 512          # d_state
P = 128          # SBUF partitions
NCORES = 8
SS = S // 2      # per-core sequence shard (4096)
NT = SS // P     # 32 t-tiles per core
CH = 4           # DMA chunks per tensor
TPC = NT // CH   # 8 t-tiles per chunk
DH = D // 2      # 256, d-half owned per core after ReduceScatter

_GRAPH_CACHE = {}


def _decay_weights():
    # Match reference: alpha = f32(exp(-pi/S)); w = exp((S-1-t) * log(alpha)) in f32.
    alpha = np.float32(math.exp(-math.pi / S))
    t = np.arange(S, dtype=np.float32)
    w = np.exp((np.float32(S - 1.0) - t) * np.log(alpha)).astype(np.float32)
    return w


def _build():
    if "nc" in _GRAPH_CACHE:
        return _GRAPH_CACHE["nc"]

    import concourse.bass as bass  # noqa: F401
    import concourse.mybir as mybir
    import concourse.tile as tile
    from concourse import bacc

    f32 = mybir.dt.float32
    bf16 = mybir.dt.bfloat16
    AF = mybir.ActivationFunctionType

    nc = bacc.Bacc("TRN2", target_bir_lowering=False)

    h_ext = nc.declare_dram_parameter("h", [SS, D], f32, isOutput=False)
    pe_ext = nc.declare_dram_parameter("pe", [SS, D], f32, isOutput=False)
    w_ext = nc.declare_dram_parameter("W", [E, D], f32, isOutput=False)
    b_ext = nc.declare_dram_parameter("b", [E], f32, isOutput=False)
    wdec_ext = nc.declare_dram_parameter("wdec", [P, NT], f32, isOutput=False)
    out_ext = nc.declare_dram_parameter("out", [E, DH], f32, isOutput=True)

    ident = nc.inline_tensor(np.eye(P, dtype=np.float32), "ident")

    h_re = h_ext.ap().rearrange("(n p) d -> p n d", p=P)
    pe_re = pe_ext.ap().rearrange("(n p) d -> p n d", p=P)
    w_re = w_ext.ap().rearrange("(c p) e -> p c e", p=P)

    with tile.TileContext(nc) as tc:
        with (
            tc.tile_pool(name="consts", bufs=1) as consts,
            tc.tile_pool(name="io", bufs=3) as io,
            tc.tile_pool(name="acc", bufs=1, space="PSUM") as acc_pool,
            tc.tile_pool(name="pst", bufs=2, space="PSUM") as pst,
            tc.tile_pool(name="dram", bufs=1, space="DRAM") as dram,
        ):
            # ---- constants ----
            ident_sb = consts.tile([P, P], f32)
            nc.sync.dma_start(ident_sb[:], ident[:, :])
            wdec_sb = consts.tile([P, NT], f32)
            nc.sync.dma_start(wdec_sb[:], wdec_ext[:, :])
            wdec_bf = consts.tile([P, NT], bf16)
            nc.vector.tensor_copy(wdec_bf[:], wdec_sb[:])
            b_sb = consts.tile([1, E], f32)
            nc.sync.dma_start(b_sb[:], b_ext.ap().unsqueeze(0))
            b_bf = consts.tile([1, E], bf16)
            nc.vector.tensor_copy(b_bf[:], b_sb[:])

            # ---- W load + on-device transpose (W^T cached in SBUF, bf16) ----
            w_sb = consts.tile([P, 4, D], f32)      # w_sb[p, c, e] = W[c*128+p, e]
            nc.sync.dma_start(w_sb[:], w_re)
            wt_sb = consts.tile([P, 4, E], bf16)    # wt_sb[p, c, s] = W[s, c*128+p]
            for ce in range(4):
                for cs in range(4):
                    tp = pst.tile([P, P], f32, tag="tp")
                    nc.tensor.transpose(
                        tp[:], w_sb[:, cs, ce * P:(ce + 1) * P], ident_sb[:]
                    )
                    nc.vector.tensor_copy(wt_sb[:, ce, cs * P:(cs + 1) * P], tp[:])

            # ---- G / r accumulation over the 4096-token shard ----
            g_ps = [
                acc_pool.tile([P, D], f32, tag=f"g{k}", name=f"g_ps{k}")
                for k in range(4)
            ]
            r_ps = acc_pool.tile([1, D], f32, tag="r")

            for j in range(CH):
                h_t = io.tile([P, TPC, D], f32, tag="h")
                pe_t = io.tile([P, TPC, D], f32, tag="pe")
                hw_t = io.tile([P, TPC, D], bf16, tag="hw")
                pew_t = io.tile([P, TPC, D], bf16, tag="pew")
                nc.sync.dma_start(h_t[:], h_re[:, j * TPC:(j + 1) * TPC, :])
                nc.sync.dma_start(pe_t[:], pe_re[:, j * TPC:(j + 1) * TPC, :])
                for i in range(TPC):
                    n = j * TPC + i
                    sc = wdec_sb[:, n:n + 1]
                    # decay-scale h -> bf16 on ACT; cast pe -> bf16 on DVE
                    nc.scalar.activation(hw_t[:, i, :], h_t[:, i, :], AF.Copy, scale=sc)
                    nc.vector.tensor_copy(pew_t[:, i, :], pe_t[:, i, :])
                    first = n == 0
                    last = n == NT - 1
                    for k in range(4):
                        nc.tensor.matmul(
                            g_ps[k][:],
                            hw_t[:, i, k * P:(k + 1) * P],
                            pew_t[:, i, :],
                            start=first,
                            stop=last,
                        )
                    nc.tensor.matmul(
                        r_ps[:],
                        wdec_bf[:, n:n + 1],
                        pew_t[:, i, :],
                        start=first,
                        stop=last,
                    )

            # ---- pack [G; r] halves into the ReduceScatter input layout ----
            # cc_in rows: [0:513) = [G | r] d-cols 0:256 (even core's half),
            #             [513:1026) = d-cols 256:512 (odd core's half).
            cc_in = dram.tile([2 * (E + 1), DH], f32)
            cc_out = dram.tile([E + 1, DH], f32)
            for k in range(4):
                g_sb = io.tile([P, D], f32, tag="gsb")
                nc.vector.tensor_copy(g_sb[:], g_ps[k][:])
                nc.sync.dma_start(cc_in[k * P:(k + 1) * P, :], g_sb[:, 0:DH])
                nc.sync.dma_start(
                    cc_in[(E + 1) + k * P:(E + 1) + (k + 1) * P, :], g_sb[:, DH:D]
                )
            r_sb = io.tile([1, D], f32, tag="rsb")
            nc.vector.tensor_copy(r_sb[:], r_ps[:])
            nc.sync.dma_start(cc_in[E:E + 1, :], r_sb[:, 0:DH])
            nc.sync.dma_start(cc_in[2 * E + 1:2 * E + 2, :], r_sb[:, DH:D])

            nc.gpsimd.collective_compute(
                "ReduceScatter",
                mybir.AluOpType.add,
                replica_groups=[[0, 1], [2, 3], [4, 5], [6, 7]],
                ins=[cc_in.opt()],
                outs=[cc_out.opt()],
            )

            # ---- finish C_half = W @ G_half + b outer r_half ----
            gred = consts.tile([P, 4, DH], f32)
            nc.sync.dma_start(
                gred[:], cc_out[0:E, :].rearrange("(c p) d -> p c d", p=P)
            )
            rred = consts.tile([1, DH], f32)
            nc.sync.dma_start(rred[:], cc_out[E:E + 1, :])
            gred_bf = consts.tile([P, 4, DH], bf16)
            nc.vector.tensor_copy(gred_bf[:], gred[:])
            rred_bf = consts.tile([1, DH], bf16)
            nc.vector.tensor_copy(rred_bf[:], rred[:])

            for cs in range(4):
                c_ps = pst.tile([P, DH], f32, tag="cps", bufs=1)
                for ce in range(4):
                    nc.tensor.matmul(
                        c_ps[:],
                        wt_sb[:, ce, cs * P:(cs + 1) * P],
                        gred_bf[:, ce, :],
                        start=(ce == 0),
                        stop=False,
                    )
                nc.tensor.matmul(
                    c_ps[:],
                    b_bf[0:1, cs * P:(cs + 1) * P],
                    rred_bf[:],
                    start=False,
                    stop=True,
                )
                c_sb = io.tile([P, DH], f32, tag="csb")
                nc.vector.tensor_copy(c_sb[:], c_ps[:])
                nc.sync.dma_start(out_ext[cs * P:(cs + 1) * P, :], c_sb[:])

    nc.compile()
    _GRAPH_CACHE["nc"] = nc
    return nc


def _in_maps(hidden_states, positional_encodings, W, b):
    w_full = _decay_weights()
    wdec_halves = [
        np.ascontiguousarray(
            w_full[h * SS:(h + 1) * SS].reshape(NT, P).T, dtype=np.float32
        )
        for h in range(2)
    ]
    maps = []
    for c in range(NCORES):
        bi, half = c // 2, c % 2
        maps.append(
            {
                "h": np.ascontiguousarray(
                    hidden_states[bi, half * SS:(half + 1) * SS, :], dtype=np.float32
                ),
                "pe": np.ascontiguousarray(
                    positional_encodings[bi, half * SS:(half + 1) * SS, :],
                    dtype=np.float32,
                ),
                "W": np.ascontiguousarray(W, dtype=np.float32),
                "b": np.ascontiguousarray(b, dtype=np.float32),
                "wdec": wdec_halves[half],
            }
        )
    return maps


def _assemble(results):
    out = np.empty((B, E, D), dtype=np.float32)
    for c in range(NCORES):
        bi, half = c // 2, c % 2
        out[bi, :, half * DH:(half + 1) * DH] = results[c]["out"]
    return out


def run(hidden_states, positional_encodings, W, b, trace=False, **trace_kwargs):
    from concourse.bass_utils import run_bass_kernel_spmd

    nc = _build()
    maps = _in_maps(hidden_states, positional_encodings, W, b)
    res = run_bass_kernel_spmd(
        nc, maps, core_ids=list(range(NCORES)), trace=trace, **trace_kwargs
    )
    return _assemble(res.results), res


def kernel(hidden_states, positional_encodings, W, b):
    out, _ = run(hidden_states, positional_encodings, W, b, trace=False)
    return out


# revision 6
# speedup vs baseline: 1.2788x; 1.2788x over previous
"""Trainium2 Bass kernel for nn_AttractorState (decay-weighted outer-product state).

Reference computation (per batch b):
    C[b] = sum_t alpha^(S-1-t) * (W @ h_t + bias) outer e_t        (S = 8192)

Refactored to avoid materializing the projection and to keep the big
contraction over t in natural [t, d] layout:
    G[b]  = (w . H[b])^T @ PE[b]          # [d_model=512, d_model=512], w_t = alpha^(S-1-t)
    r[b]  = w^T @ PE[b]                   # [512]
    C[b]  = W @ G[b] + bias outer r[b]    # [d_state=512, d_model=512]

Sharding over 8 NeuronCores: (batch=4) x (d-half=2), fully collective-free.
Each core processes ALL 8192 tokens of one batch but only its 256 d-columns
of PE: G_half = (w . H)^T @ PE[:, dhalf] accumulates locally in PSUM, then
C_half = W @ G_half + b outer r_half, writing a (512, 256) slab.  The host
reassembles (4, 512, 512).  The t-contraction never crosses cores, so no
reduction, no collectives, no inter-core sync.

Matmul operands are cast to bf16 on-chip (DVE/ACT, overlapped with DMA);
accumulation is fp32 in PSUM.
"""

import math
import sys

import numpy as np

for _p in ("/opt/trn_rl_repo", "/opt/trn_rl_repo/concourse"):
    if _p not in sys.path:
        sys.path.append(_p)

# Problem constants (hardcoded per harness contract).
B = 4
S = 8192
D = 512          # d_model
E = 512          # d_state
P = 128          # SBUF partitions
NCORES = 8
DH = D // 2      # 256, d-half owned per core
NT = S // P      # 64 t-tiles per core
CH = 8           # DMA chunks per tensor
TPC = NT // CH   # 8 t-tiles per chunk

_GRAPH_CACHE = {}


def _decay_weights():
    # Match reference: alpha = f32(exp(-pi/S)); w = exp((S-1-t) * log(alpha)) in f32.
    alpha = np.float32(math.exp(-math.pi / S))
    t = np.arange(S, dtype=np.float32)
    w = np.exp((np.float32(S - 1.0) - t) * np.log(alpha)).astype(np.float32)
    return w


def _build():
    if "nc" in _GRAPH_CACHE:
        return _GRAPH_CACHE["nc"]

    import concourse.bass as bass  # noqa: F401
    import concourse.mybir as mybir
    import concourse.tile as tile
    from concourse import bacc

    f32 = mybir.dt.float32
    bf16 = mybir.dt.bfloat16
    AF = mybir.ActivationFunctionType

    nc = bacc.Bacc("TRN2", target_bir_lowering=False)

    h_ext = nc.declare_dram_parameter("h", [S, D], f32, isOutput=False)
    pe_ext = nc.declare_dram_parameter("pe", [S, DH], f32, isOutput=False)
    w_ext = nc.declare_dram_parameter("W", [E, D], f32, isOutput=False)
    b_ext = nc.declare_dram_parameter("b", [E], f32, isOutput=False)
    wdec_ext = nc.declare_dram_parameter("wdec", [P, NT], f32, isOutput=False)
    out_ext = nc.declare_dram_parameter("out", [E, DH], f32, isOutput=True)

    ident = nc.inline_tensor(np.eye(P, dtype=np.float32), "ident")

    h_re = h_ext.ap().rearrange("(n p) d -> p n d", p=P)
    pe_re = pe_ext.ap().rearrange("(n p) d -> p n d", p=P)
    w_re = w_ext.ap().rearrange("(c p) e -> p c e", p=P)

    with tile.TileContext(nc) as tc:
        with (
            tc.tile_pool(name="consts", bufs=1) as consts,
            tc.tile_pool(name="io", bufs=3) as io,
            tc.tile_pool(name="acc", bufs=1, space="PSUM") as acc_pool,
            tc.tile_pool(name="pst", bufs=2, space="PSUM") as pst,
        ):
            # ---- constants ----
            ident_sb = consts.tile([P, P], f32)
            nc.sync.dma_start(ident_sb[:], ident[:, :])
            wdec_sb = consts.tile([P, NT], f32)
            nc.sync.dma_start(wdec_sb[:], wdec_ext[:, :])
            wdec_bf = consts.tile([P, NT], bf16)
            nc.vector.tensor_copy(wdec_bf[:], wdec_sb[:])
            b_sb = consts.tile([1, E], f32)
            nc.sync.dma_start(b_sb[:], b_ext.ap().unsqueeze(0))
            b_bf = consts.tile([1, E], bf16)
            nc.vector.tensor_copy(b_bf[:], b_sb[:])

            # ---- W load + on-device transpose (W^T cached in SBUF, bf16) ----
            w_sb = consts.tile([P, 4, D], f32)      # w_sb[p, c, e] = W[c*128+p, e]
            nc.sync.dma_start(w_sb[:], w_re)
            wt_sb = consts.tile([P, 4, E], bf16)    # wt_sb[p, c, s] = W[s, c*128+p]
            for ce in range(4):
                for cs in range(4):
                    tp = pst.tile([P, P], f32, tag="tp")
                    nc.tensor.transpose(
                        tp[:], w_sb[:, cs, ce * P:(ce + 1) * P], ident_sb[:]
                    )
                    nc.vector.tensor_copy(wt_sb[:, ce, cs * P:(cs + 1) * P], tp[:])

            # ---- G / r accumulation over all 8192 tokens ----
            g_ps = [
                acc_pool.tile([P, DH], f32, tag=f"g{k}", name=f"g_ps{k}")
                for k in range(4)
            ]
            r_ps = acc_pool.tile([1, DH], f32, tag="r")

            for j in range(CH):
                h_t = io.tile([P, TPC, D], f32, tag="h")
                pe_t = io.tile([P, TPC, DH], f32, tag="pe")
                hw_t = io.tile([P, TPC, D], bf16, tag="hw")
                pew_t = io.tile([P, TPC, DH], bf16, tag="pew")
                nc.sync.dma_start(h_t[:], h_re[:, j * TPC:(j + 1) * TPC, :])
                nc.sync.dma_start(pe_t[:], pe_re[:, j * TPC:(j + 1) * TPC, :])
                for i in range(TPC):
                    n = j * TPC + i
                    sc = wdec_sb[:, n:n + 1]
                    # decay-scale h -> bf16, cast pe -> bf16; alternate engines
                    if i % 2 == 0:
                        nc.scalar.activation(hw_t[:, i, :], h_t[:, i, :], AF.Copy, scale=sc)
                        nc.vector.tensor_copy(pew_t[:, i, :], pe_t[:, i, :])
                    else:
                        nc.vector.tensor_scalar_mul(hw_t[:, i, :], h_t[:, i, :], sc)
                        nc.scalar.activation(pew_t[:, i, :], pe_t[:, i, :], AF.Copy)
                    first = n == 0
                    last = n == NT - 1
                    for k in range(4):
                        nc.tensor.matmul(
                            g_ps[k][:],
                            hw_t[:, i, k * P:(k + 1) * P],
                            pew_t[:, i, :],
                            start=first,
                            stop=last,
                        )
                    nc.tensor.matmul(
                        r_ps[:],
                        wdec_bf[:, n:n + 1],
                        pew_t[:, i, :],
                        start=first,
                        stop=last,
                    )

            # ---- G -> SBUF (bf16) ----
            gred_bf = consts.tile([P, 4, DH], bf16)
            for k in range(4):
                nc.vector.tensor_copy(gred_bf[:, k, :], g_ps[k][:])
            rred_bf = consts.tile([1, DH], bf16)
            nc.vector.tensor_copy(rred_bf[:], r_ps[:])

            # ---- finish C_half = W @ G_half + b outer r_half ----
            for cs in range(4):
                c_ps = pst.tile([P, DH], f32, tag="cps", bufs=1)
                for ce in range(4):
                    nc.tensor.matmul(
                        c_ps[:],
                        wt_sb[:, ce, cs * P:(cs + 1) * P],
                        gred_bf[:, ce, :],
                        start=(ce == 0),
                        stop=False,
                    )
                nc.tensor.matmul(
                    c_ps[:],
                    b_bf[0:1, cs * P:(cs + 1) * P],
                    rred_bf[:],
                    start=False,
                    stop=True,
                )
                c_sb = io.tile([P, DH], f32, tag="csb")
                nc.vector.tensor_copy(c_sb[:], c_ps[:])
                nc.sync.dma_start(out_ext[cs * P:(cs + 1) * P, :], c_sb[:])

    nc.compile()
    _GRAPH_CACHE["nc"] = nc
    return nc


def _in_maps(hidden_states, positional_encodings, W, b):
    w_full = _decay_weights()
    wdec = np.ascontiguousarray(w_full.reshape(NT, P).T, dtype=np.float32)
    W_c = np.ascontiguousarray(W, dtype=np.float32)
    b_c = np.ascontiguousarray(b, dtype=np.float32)
    maps = []
    for c in range(NCORES):
        bi, dh = c // 2, c % 2
        maps.append(
            {
                "h": np.ascontiguousarray(hidden_states[bi], dtype=np.float32),
                "pe": np.ascontiguousarray(
                    positional_encodings[bi, :, dh * DH:(dh + 1) * DH],
                    dtype=np.float32,
                ),
                "W": W_c,
                "b": b_c,
                "wdec": wdec,
            }
        )
    return maps


def _assemble(results):
    out = np.empty((B, E, D), dtype=np.float32)
    for c in range(NCORES):
        bi, dh = c // 2, c % 2
        out[bi, :, dh * DH:(dh + 1) * DH] = results[c]["out"]
    return out


def run(hidden_states, positional_encodings, W, b, trace=False, **trace_kwargs):
    from concourse.bass_utils import run_bass_kernel_spmd

    nc = _build()
    maps = _in_maps(hidden_states, positional_encodings, W, b)
    res = run_bass_kernel_spmd(
        nc, maps, core_ids=list(range(NCORES)), trace=trace, **trace_kwargs
    )
    return _assemble(res.results), res


def kernel(hidden_states, positional_encodings, W, b):
    out, _ = run(hidden_states, positional_encodings, W, b, trace=False)
    return out
